# revision 8
# baseline (speedup 1.0000x reference)
"""Masked (expander) linear layer on 8 Trainium2 NeuronCores.

Computes out = x @ (W * M)^T for
  x: [16384, 2048] f32, W: [2048, 2048] f32, M: [2048, 2048] int32 (0/1)

Sharding: pure data-parallel over rows of x. Each of the 8 cores gets 2048
rows of x plus a replicated (transposed) copy of W and M, computes its
[2048, 2048] output shard entirely locally, and the host concatenates
shards. No collectives.

Device-side design (v2, bf16 all-resident):
 - x and W are repacked to bf16 on the host (same rounding a device-side
   cast-DMA would apply; 1.35e-4 -> 2.1e-3 rel err, far under tolerance),
   the mask to int8. Input HBM traffic per core drops 36MB -> 20MB. The
   mask multiply (the module's elementwise FLOPs) still runs on DVE; all
   matmul FLOPs run on PE.
 - Everything is SBUF-resident: wm (masked weight, 4 n-chunks x 4
   k-quarter tiles, bf16, 64KB/partition) and x (4 m-blocks x 4
   k-quarters, bf16, 64KB/partition). x is loaded exactly once; no JIT
   re-streaming.
 - All input DMAs ride the sync HWDGE ring in exact PE consumption
   order: x block0 quarters interleaved with (mask, W) quarters of
   n-chunk 0, then n-chunks 1-3, then x blocks 1-3. y evacuations ride
   the scalar ring. Arrival therefore tracks the PE's needs: the PE
   starts ~6us in and is paced by the W stream only through the first
   row of (n-chunk, block) pairs.
 - PE order: pairs (nt, b=0) for nt 0..3 first (paced by the W stream),
   then nt-outer over blocks 1..3 (everything resident by then).
 - A short burst of tiny warm-up matmuls on a scratch PSUM bank runs
   while the first DMAs land, so the HAM clock-gate is at full rate
   (2.4 GHz) before the first real matmul issues.
 - PSUM groups rotate over all 8 banks; each group is evacuated
   (ScalarE copy + scalar-ring DMA) right after it closes.
"""

from contextlib import ExitStack

import numpy as np
import ml_dtypes

import concourse.bacc as bacc
import concourse.bass as bass
import concourse.mybir as mybir
import concourse.tile as tile
from concourse.bass_utils import run_bass_kernel_spmd

N_CORES = 8
P = 128

FULL_N, FULL_OUT, FULL_IN = 16384, 2048, 2048


def build_nc(
    rows: int = FULL_N // N_CORES,
    in_dim: int = FULL_IN,
    out_dim: int = FULL_OUT,
    n_chunk: int = 512,
    m_block: int = 4,
    warmup_mms: int = 72,
):
    """Per-core Bass module: y[rows, out] = x @ (wt * m), bf16 inputs.

    DRAM layouts: wt/mk panel-major [NT, in_dim, n_chunk] (wt bf16, mk
    int8); x transposed [in_dim, rows] bf16; y row-major [rows, out_dim]
    f32.
    """
    assert rows % P == 0 and in_dim % P == 0 and out_dim % n_chunk == 0
    KT = in_dim // P
    MT = rows // P
    NT = out_dim // n_chunk
    assert KT % 4 == 0 and MT % m_block == 0
    KQ = KT // 4
    NB = MT // m_block
    mw = m_block * P  # columns of x per block

    bf16 = mybir.dt.bfloat16

    nc = bacc.Bacc("TRN2", target_bir_lowering=False, debug=False)
    x = nc.dram_tensor("x", [in_dim, rows], bf16, kind="ExternalInput")
    wt = nc.dram_tensor("wt", [NT, in_dim, n_chunk], bf16, kind="ExternalInput")
    mk = nc.dram_tensor("mk", [NT, in_dim, n_chunk], mybir.dt.int8, kind="ExternalInput")
    y = nc.dram_tensor("y", [rows, out_dim], mybir.dt.float32, kind="ExternalOutput")

    # K-major DRAM views: [.., p, kt, ..]
    wt_v = wt[:, :, :].rearrange("t (kt p) n -> t p kt n", p=P)
    mk_v = mk[:, :, :].rearrange("t (kt p) n -> t p kt n", p=P)
    x_v = x[:, :].rearrange("(kt p) m -> p kt m", p=P)

    with ExitStack() as ctx:
        tc = ctx.enter_context(tile.TileContext(nc))
        wm_pool = ctx.enter_context(tc.tile_pool(name="wm", bufs=1))
        xt_pool = ctx.enter_context(tc.tile_pool(name="xt", bufs=1))
        ws_pool = ctx.enter_context(tc.tile_pool(name="ws", bufs=4))
        msk_pool = ctx.enter_context(tc.tile_pool(name="msk", bufs=4))
        yo_pool = ctx.enter_context(tc.tile_pool(name="yo", bufs=3))
        wu_pool = ctx.enter_context(tc.tile_pool(name="wu", bufs=1))
        pm_pool = ctx.enter_context(tc.tile_pool(name="pm", bufs=1, space="PSUM"))

        # Resident masked weight: wm_t[nt][q] of shape [P, KQ, n_chunk] bf16.
        # The very first quarter (nt=0, q=0) is split into KQ single-ktile
        # tiles so the first matmul's dependency is ~320KB of DMA, not
        # 1.25MB — the PE starts ~6us earlier.
        wm_t = [
            [
                wm_pool.tile([P, KQ, n_chunk], bf16, tag=f"wm{nt}_{q}", name=f"wm{nt}_{q}")
                if not (nt == 0 and q == 0)
                else None
                for q in range(4)
            ]
            for nt in range(NT)
        ]
        wm00 = [
            wm_pool.tile([P, 1, n_chunk], bf16, tag=f"wm00k{k}", name=f"wm00k{k}")
            for k in range(KQ)
        ]
        # Resident x: xt_t[b][q] of shape [P, KQ, m_block*P] bf16 (b=0, q=0
        # likewise split per ktile)
        xt_t = [
            [
                xt_pool.tile([P, KQ, mw], bf16, tag=f"xt{b}_{q}", name=f"xt{b}_{q}")
                if not (b == 0 and q == 0)
                else None
                for q in range(4)
            ]
            for b in range(NB)
        ]
        xt00 = [
            xt_pool.tile([P, 1, mw], bf16, tag=f"xt00k{k}", name=f"xt00k{k}")
            for k in range(KQ)
        ]

        def x_ap(b, q, k):
            if b == 0 and q == 0:
                return xt00[k][:, 0, :]
            return xt_t[b][q][:, k, :]

        def wm_ap(nt, q, k):
            if nt == 0 and q == 0:
                return wm00[k][:, 0, :]
            return wm_t[nt][q][:, k, :]

        # ---- PE warm-up: tiny matmuls on scratch data keep the HAM
        # activity window busy while the first input DMAs land, so real
        # matmuls start at the full 2.4 GHz clock. Bank 7 is not needed
        # by real groups until pair index 1, long after these drain.
        if warmup_mms:
            wu = wu_pool.tile([P, P], bf16, tag="wu", name="wu")
            nc.vector.memset(wu[:], 0.0)
            pwu = pm_pool.tile([P, 64], mybir.dt.float32, tag="pm7", name="pmwu")
            for i in range(warmup_mms):
                nc.tensor.matmul(pwu[:], wu[:], wu[:, :64], start=True, stop=True)

        def load_x_piece(b, q):
            ksl = slice(q * KQ, (q + 1) * KQ)
            nc.sync.dma_start(
                out=xt_t[b][q][:], in_=x_v[:, ksl, b * mw : (b + 1) * mw]
            )

        def load_w_piece(nt, q):
            ksl = slice(q * KQ, (q + 1) * KQ)
            mtile = msk_pool.tile([P, KQ, n_chunk], mybir.dt.int8, tag="mt")
            nc.sync.dma_start(out=mtile[:], in_=mk_v[nt, :, ksl, :])
            wstage = ws_pool.tile([P, KQ, n_chunk], bf16, tag="ws")
            nc.sync.dma_start(out=wstage[:], in_=wt_v[nt, :, ksl, :])
            # masked multiply on DVE (bf16: 2x throughput), one op per piece
            nc.vector.tensor_mul(wm_t[nt][q][:], wstage[:], mtile[:])

        # ---- input stream, in exact PE consumption order, all on the
        # sync HWDGE ring (FIFO): the first quarter per single ktile,
        # x0/nt0 interleaved, nt1-3, x1-3.
        for k in range(KQ):
            nc.sync.dma_start(out=xt00[k][:], in_=x_v[:, k : k + 1, 0:mw])
            mtile = msk_pool.tile([P, 1, n_chunk], mybir.dt.int8, tag="mt0")
            nc.sync.dma_start(out=mtile[:], in_=mk_v[0, :, k : k + 1, :])
            wstage = ws_pool.tile([P, 1, n_chunk], bf16, tag="ws0")
            nc.sync.dma_start(out=wstage[:], in_=wt_v[0, :, k : k + 1, :])
            nc.vector.tensor_mul(wm00[k][:], wstage[:], mtile[:])
        for q in range(1, 4):
            load_x_piece(0, q)
            load_w_piece(0, q)
        for nt in range(1, NT):
            for q in range(4):
                load_w_piece(nt, q)
        for b in range(1, NB):
            for q in range(4):
                load_x_piece(b, q)

        # ---- PE: pair (nt, b) = m_block PSUM groups of KT matmuls each
        pair_idx = [0]

        n_pairs = NT * NB

        def pair(nt, b):
            g0 = pair_idx[0] * m_block
            last_pair = pair_idx[0] == n_pairs - 1
            pair_idx[0] += 1
            pms = {
                mb: pm_pool.tile(
                    [P, n_chunk],
                    mybir.dt.float32,
                    tag=f"pm{(g0 + mb) % 8}",
                    name=f"pm{(g0 + mb) % 8}",
                )
                for mb in range(m_block)
            }
            for q in range(4):
                for mb in range(m_block):
                    for k in range(KQ):
                        kt = q * KQ + k
                        nc.tensor.matmul(
                            pms[mb][:],
                            x_ap(b, q, k)[:, bass.ts(mb, P)],
                            wm_ap(nt, q, k),
                            start=(kt == 0),
                            stop=(kt == KT - 1),
                        )
                    if q == 3:
                        mt = b * m_block + mb
                        yo = yo_pool.tile([P, n_chunk], mybir.dt.float32, tag="yo")
                        if last_pair:
                            # shorten the kernel tail: evacuate in halves on
                            # DVE so the final copy+DMA chain after the last
                            # matmul is as short as possible
                            h = n_chunk // 2
                            for i in range(2):
                                sl = slice(i * h, (i + 1) * h)
                                nc.vector.tensor_copy(yo[:, sl], pms[mb][:, sl])
                                nc.scalar.dma_start(
                                    out=y[
                                        mt * P : (mt + 1) * P,
                                        nt * n_chunk + i * h : nt * n_chunk + (i + 1) * h,
                                    ],
                                    in_=yo[:, sl],
                                )
                        else:
                            nc.scalar.copy(yo[:], pms[mb][:])
                            nc.scalar.dma_start(
                                out=y[mt * P : (mt + 1) * P, bass.ts(nt, n_chunk)],
                                in_=yo[:],
                            )

        # b0 row first (paced by the W stream), then nt-outer over the rest
        for nt in range(NT):
            pair(nt, 0)
        for nt in range(NT):
            for b in range(1, NB):
                pair(nt, b)

    nc.compile()
    return nc


def _prep_host(input_, weight, mask, n_chunk=512):
    bf = ml_dtypes.bfloat16
    in_dim, out_dim = weight.shape[1], weight.shape[0]
    nt = out_dim // n_chunk
    # weight.T -> [NT, IN, n_chunk] bf16, each panel contiguous
    wtp = np.ascontiguousarray(
        weight.T.reshape(in_dim, nt, n_chunk).transpose(1, 0, 2)
    ).astype(bf)
    mkp = np.ascontiguousarray(
        mask.T.reshape(in_dim, nt, n_chunk).transpose(1, 0, 2)
    ).astype(np.int8)
    rows = input_.shape[0] // N_CORES
    in_maps = []
    for c in range(N_CORES):
        xp = input_[c * rows : (c + 1) * rows].T.astype(bf)  # contiguous copy
        in_maps.append({"x": xp, "wt": wtp, "mk": mkp})
    return in_maps


_CACHE = {}


def _run(input_, weight, mask, trace=False, **build_kw):
    rows_total, in_dim = input_.shape
    out_dim = weight.shape[0]
    key = (rows_total, in_dim, out_dim, tuple(sorted(build_kw.items())))
    if key not in _CACHE:
        _CACHE[key] = build_nc(
            rows=rows_total // N_CORES, in_dim=in_dim, out_dim=out_dim, **build_kw
        )
    nc = _CACHE[key]
    in_maps = _prep_host(input_, weight, mask, build_kw.get("n_chunk", 512))
    res = run_bass_kernel_spmd(nc, in_maps, core_ids=list(range(N_CORES)), trace=trace)
    out = np.concatenate([res.results[c]["y"] for c in range(N_CORES)], axis=0)
    return out, res


def kernel(input_, weight, mask):
    input_ = np.asarray(input_, dtype=np.float32)
    weight = np.asarray(weight, dtype=np.float32)
    mask = np.asarray(mask)
    out, _ = _run(input_, weight, mask, trace=False)
    return out


# revision 11
# speedup vs baseline: 1.1915x; 1.1915x over previous
"""Masked (expander) linear layer on 8 Trainium2 NeuronCores.

Computes out = x @ (W * M)^T for
  x: [16384, 2048] f32, W: [2048, 2048] f32, M: [2048, 2048] int32 (0/1)

Sharding: pure data-parallel over rows of x. Each of the 8 cores gets 2048
rows of x plus a replicated (transposed) copy of W and M, computes its
[2048, 2048] output shard entirely locally, and the host concatenates
shards. No collectives.

Device-side design (v2, bf16 all-resident):
 - x and W are repacked to bf16 on the host (same rounding a device-side
   cast-DMA would apply; 1.35e-4 -> 2.1e-3 rel err, far under tolerance),
   the mask to int8. Input HBM traffic per core drops 36MB -> 20MB. The
   mask multiply (the module's elementwise FLOPs) still runs on DVE; all
   matmul FLOPs run on PE.
 - Everything is SBUF-resident: wm (masked weight, 4 n-chunks x 4
   k-quarter tiles, bf16, 64KB/partition) and x (4 m-blocks x 4
   k-quarters, bf16, 64KB/partition). x is loaded exactly once; no JIT
   re-streaming.
 - All input DMAs ride the sync HWDGE ring in exact PE consumption
   order: x block0 quarters interleaved with (mask, W) quarters of
   n-chunk 0, then n-chunks 1-3, then x blocks 1-3. y evacuations ride
   the scalar ring. Arrival therefore tracks the PE's needs: the PE
   starts ~6us in and is paced by the W stream only through the first
   row of (n-chunk, block) pairs.
 - PE order: pairs (nt, b=0) for nt 0..3 first (paced by the W stream),
   then nt-outer over blocks 1..3 (everything resident by then).
 - A short burst of tiny warm-up matmuls on a scratch PSUM bank runs
   while the first DMAs land, so the HAM clock-gate is at full rate
   (2.4 GHz) before the first real matmul issues.
 - PSUM groups rotate over all 8 banks; each group is evacuated
   (ScalarE copy + scalar-ring DMA) right after it closes.
"""

from contextlib import ExitStack

import numpy as np
import ml_dtypes

import concourse.bacc as bacc
import concourse.bass as bass
import concourse.mybir as mybir
import concourse.tile as tile
from concourse.bass_utils import run_bass_kernel_spmd

N_CORES = 8
P = 128

FULL_N, FULL_OUT, FULL_IN = 16384, 2048, 2048


def build_nc(
    rows: int = FULL_N // N_CORES,
    in_dim: int = FULL_IN,
    out_dim: int = FULL_OUT,
    n_chunk: int = 512,
    m_block: int = 4,
    warmup_mms: int = 56,
):
    """Per-core Bass module: y[rows, out] = x @ (wt * m), bf16 inputs.

    DRAM layouts: wt/mk panel-major [NT, in_dim, n_chunk] (wt bf16, mk
    int8); x transposed [in_dim, rows] bf16; y row-major [rows, out_dim]
    f32.
    """
    assert rows % P == 0 and in_dim % P == 0 and out_dim % n_chunk == 0
    KT = in_dim // P
    MT = rows // P
    NT = out_dim // n_chunk
    assert KT % 4 == 0 and MT % m_block == 0
    KQ = KT // 4
    NB = MT // m_block
    mw = m_block * P  # columns of x per block

    bf16 = mybir.dt.bfloat16

    nc = bacc.Bacc("TRN2", target_bir_lowering=False, debug=False)
    x = nc.dram_tensor("x", [in_dim, rows], bf16, kind="ExternalInput")
    wt = nc.dram_tensor("wt", [NT, in_dim, n_chunk], bf16, kind="ExternalInput")
    mk = nc.dram_tensor("mk", [NT, in_dim, n_chunk], mybir.dt.int8, kind="ExternalInput")
    y = nc.dram_tensor("y", [rows, out_dim], mybir.dt.float32, kind="ExternalOutput")

    # K-major DRAM views: [.., p, kt, ..]
    wt_v = wt[:, :, :].rearrange("t (kt p) n -> t p kt n", p=P)
    mk_v = mk[:, :, :].rearrange("t (kt p) n -> t p kt n", p=P)
    x_v = x[:, :].rearrange("(kt p) m -> p kt m", p=P)

    with ExitStack() as ctx:
        tc = ctx.enter_context(tile.TileContext(nc))
        wm_pool = ctx.enter_context(tc.tile_pool(name="wm", bufs=1))
        xt_pool = ctx.enter_context(tc.tile_pool(name="xt", bufs=1))
        ws_pool = ctx.enter_context(tc.tile_pool(name="ws", bufs=4))
        msk_pool = ctx.enter_context(tc.tile_pool(name="msk", bufs=4))
        yo_pool = ctx.enter_context(tc.tile_pool(name="yo", bufs=3))
        wu_pool = ctx.enter_context(tc.tile_pool(name="wu", bufs=1))
        pm_pool = ctx.enter_context(tc.tile_pool(name="pm", bufs=1, space="PSUM"))

        # Resident masked weight: wm_t[nt][q] of shape [P, KQ, n_chunk] bf16.
        # The very first quarter (nt=0, q=0) is split into KQ single-ktile
        # tiles so the first matmul's dependency is ~320KB of DMA, not
        # 1.25MB — the PE starts ~6us earlier.
        wm_t = [
            [
                wm_pool.tile([P, KQ, n_chunk], bf16, tag=f"wm{nt}_{q}", name=f"wm{nt}_{q}")
                if not (nt == 0 and q == 0)
                else None
                for q in range(4)
            ]
            for nt in range(NT)
        ]
        wm00 = [
            wm_pool.tile([P, 1, n_chunk], bf16, tag=f"wm00k{k}", name=f"wm00k{k}")
            for k in range(KQ)
        ]
        # Resident x: xt_t[b][q] of shape [P, KQ, m_block*P] bf16 (b=0, q=0
        # likewise split per ktile)
        xt_t = [
            [
                xt_pool.tile([P, KQ, mw], bf16, tag=f"xt{b}_{q}", name=f"xt{b}_{q}")
                if not (b == 0 and q == 0)
                else None
                for q in range(4)
            ]
            for b in range(NB)
        ]
        xt00 = [
            xt_pool.tile([P, 1, mw], bf16, tag=f"xt00k{k}", name=f"xt00k{k}")
            for k in range(KQ)
        ]

        def x_ap(b, q, k):
            if b == 0 and q == 0:
                return xt00[k][:, 0, :]
            return xt_t[b][q][:, k, :]

        def wm_ap(nt, q, k):
            if nt == 0 and q == 0:
                return wm00[k][:, 0, :]
            return wm_t[nt][q][:, k, :]

        # ---- PE warm-up: tiny matmuls on scratch data keep the HAM
        # activity window busy while the first input DMAs land, so real
        # matmuls start at the full 2.4 GHz clock. Bank 7 is not needed
        # by real groups until pair index 1, long after these drain.
        if warmup_mms:
            wu = wu_pool.tile([P, P], bf16, tag="wu", name="wu")
            nc.vector.memset(wu[:], 0.0)
            pwu = pm_pool.tile([P, 64], mybir.dt.float32, tag="pm7", name="pmwu")
            for i in range(warmup_mms):
                nc.tensor.matmul(pwu[:], wu[:], wu[:, :64], start=True, stop=True)

        def load_x_piece(b, q):
            ksl = slice(q * KQ, (q + 1) * KQ)
            nc.sync.dma_start(
                out=xt_t[b][q][:], in_=x_v[:, ksl, b * mw : (b + 1) * mw]
            )

        def load_w_piece(nt, q):
            ksl = slice(q * KQ, (q + 1) * KQ)
            mtile = msk_pool.tile([P, KQ, n_chunk], mybir.dt.int8, tag="mt")
            nc.sync.dma_start(out=mtile[:], in_=mk_v[nt, :, ksl, :])
            wstage = ws_pool.tile([P, KQ, n_chunk], bf16, tag="ws")
            nc.sync.dma_start(out=wstage[:], in_=wt_v[nt, :, ksl, :])
            # masked multiply on DVE (bf16: 2x throughput), one op per piece
            nc.vector.tensor_mul(wm_t[nt][q][:], wstage[:], mtile[:])

        # ---- input stream, in exact PE consumption order, all on the
        # sync HWDGE ring (FIFO): the first quarter per single ktile,
        # x0/nt0 interleaved, then the remaining x blocks (cheap 2MB
        # unlocks for pairs (0,1..3)), then nt1-3.
        for k in range(KQ):
            nc.sync.dma_start(out=xt00[k][:], in_=x_v[:, k : k + 1, 0:mw])
            mtile = msk_pool.tile([P, 1, n_chunk], mybir.dt.int8, tag="mt0")
            nc.sync.dma_start(out=mtile[:], in_=mk_v[0, :, k : k + 1, :])
            wstage = ws_pool.tile([P, 1, n_chunk], bf16, tag="ws0")
            nc.sync.dma_start(out=wstage[:], in_=wt_v[0, :, k : k + 1, :])
            nc.vector.tensor_mul(wm00[k][:], wstage[:], mtile[:])
        for q in range(1, 4):
            load_x_piece(0, q)
            load_w_piece(0, q)
        for b in range(1, NB):
            for q in range(4):
                load_x_piece(b, q)
        for nt in range(1, NT):
            for q in range(4):
                load_w_piece(nt, q)

        # ---- PE: pair (nt, b) = m_block PSUM groups of KT matmuls each
        pair_idx = [0]

        n_pairs = NT * NB

        def pair(nt, b):
            g0 = pair_idx[0] * m_block
            last_pair = pair_idx[0] == n_pairs - 1
            pair_idx[0] += 1
            pms = {
                mb: pm_pool.tile(
                    [P, n_chunk],
                    mybir.dt.float32,
                    tag=f"pm{(g0 + mb) % 8}",
                    name=f"pm{(g0 + mb) % 8}",
                )
                for mb in range(m_block)
            }
            for q in range(4):
                for mb in range(m_block):
                    for k in range(KQ):
                        kt = q * KQ + k
                        nc.tensor.matmul(
                            pms[mb][:],
                            x_ap(b, q, k)[:, bass.ts(mb, P)],
                            wm_ap(nt, q, k),
                            start=(kt == 0),
                            stop=(kt == KT - 1),
                        )
                    if q == 3:
                        mt = b * m_block + mb
                        yo = yo_pool.tile([P, n_chunk], mybir.dt.float32, tag="yo")
                        if last_pair:
                            # shorten the kernel tail: evacuate in halves on
                            # DVE so the final copy+DMA chain after the last
                            # matmul is as short as possible
                            h = n_chunk // 2
                            for i in range(2):
                                sl = slice(i * h, (i + 1) * h)
                                nc.vector.tensor_copy(yo[:, sl], pms[mb][:, sl])
                                nc.scalar.dma_start(
                                    out=y[
                                        mt * P : (mt + 1) * P,
                                        nt * n_chunk + i * h : nt * n_chunk + (i + 1) * h,
                                    ],
                                    in_=yo[:, sl],
                                )
                        else:
                            nc.scalar.copy(yo[:], pms[mb][:])
                            nc.scalar.dma_start(
                                out=y[mt * P : (mt + 1) * P, bass.ts(nt, n_chunk)],
                                in_=yo[:],
                            )

        # nt-rows: the nt0 row is unlocked by the cheap x blocks while
        # nt1-3 weights stream far ahead of when the PE reaches them
        for nt in range(NT):
            for b in range(NB):
                pair(nt, b)

    nc.compile()
    return nc


def _prep_host(input_, weight, mask, n_chunk=512):
    bf = ml_dtypes.bfloat16
    in_dim, out_dim = weight.shape[1], weight.shape[0]
    nt = out_dim // n_chunk
    # weight.T -> [NT, IN, n_chunk] bf16, each panel contiguous
    wtp = np.ascontiguousarray(
        weight.T.reshape(in_dim, nt, n_chunk).transpose(1, 0, 2)
    ).astype(bf)
    mkp = np.ascontiguousarray(
        mask.T.reshape(in_dim, nt, n_chunk).transpose(1, 0, 2)
    ).astype(np.int8)
    rows = input_.shape[0] // N_CORES
    in_maps = []
    for c in range(N_CORES):
        xp = input_[c * rows : (c + 1) * rows].T.astype(bf)  # contiguous copy
        in_maps.append({"x": xp, "wt": wtp, "mk": mkp})
    return in_maps


_CACHE = {}


def _run(input_, weight, mask, trace=False, **build_kw):
    rows_total, in_dim = input_.shape
    out_dim = weight.shape[0]
    key = (rows_total, in_dim, out_dim, tuple(sorted(build_kw.items())))
    if key not in _CACHE:
        _CACHE[key] = build_nc(
            rows=rows_total // N_CORES, in_dim=in_dim, out_dim=out_dim, **build_kw
        )
    nc = _CACHE[key]
    in_maps = _prep_host(input_, weight, mask, build_kw.get("n_chunk", 512))
    res = run_bass_kernel_spmd(nc, in_maps, core_ids=list(range(N_CORES)), trace=trace)
    out = np.concatenate([res.results[c]["y"] for c in range(N_CORES)], axis=0)
    return out, res


def kernel(input_, weight, mask):
    input_ = np.asarray(input_, dtype=np.float32)
    weight = np.asarray(weight, dtype=np.float32)
    mask = np.asarray(mask)
    out, _ = _run(input_, weight, mask, trace=False)
    return out
